# revision 1
# baseline (speedup 1.0000x reference)
"""Joint-entropy (KDE logsumexp over 3x3 windows) Trainium2 kernel.

Math: for each 3x3 window of pixel vectors v_n (C=3 channels),
  out[i,j] = log_norm - (1/9) * sum_n log(S_n),  S_n = sum_m exp(-2*||v_n-v_m||^2)
with log_norm = log(9) + 3*log(sqrt(2*pi)*0.5)  (h = 0.5, logits = -2*d2).

Sharding: 8 cores = 4 batches x 2 row-halves. Each core gets a host-padded
x[b,:,r0:r0+129,:] slice and produces a [127, 254] output slab. All window
math is local (halo rows included in the input slice), so no collectives.

On-chip layout: partitions = window rows (127), free dim = image cols
(padded by 2 on both sides so column-shifted reads stay in-bounds).
All row shifts are realized by loading 3 row-shifted copies of the input
(X[c,s][p,w] = x[c, p+s, w]); every other access is a free-dim (column)
shift, so no partition-shifted operands are needed anywhere.

E-map classes (s = window row of the anchor pixel, a = row gap, b = col gap):
  (s,0) s=0,1,2 with b in {1,2};  (s,1) s=0,1 with b in {-2..2};  (0,2) b in {-2..2}.
Each class is one [127, nb, 256] tile of exp(-2*d2) values, computed with
"wide" ops covering all nb column-gaps at once (stride-0 broadcast on the
anchor operand). S_n sums then read these at column offsets 0..2 only.
"""

import dataclasses

import numpy as np

import concourse.bacc as bacc
import concourse.bass as bass
import concourse.tile as tile
from concourse import mybir
from concourse.bass_utils import run_bass_kernel_spmd

F32 = mybir.dt.float32
BF16 = mybir.dt.bfloat16
AOP = mybir.AluOpType
AF = mybir.ActivationFunctionType

C = 3
W = 256
B = 4
R = 3
ROWS_IN = 129  # 127 window rows need input rows r0 .. r0+128
ROWS_OUT = 127
WOUT = 254
PAD = 2
WT = W + 2 * PAD  # padded width (host-padded)
LOG_NORM = float(np.log(9.0) + 3.0 * np.log(np.sqrt(2.0 * np.pi) * 0.5))

# (s, a, bmin, nb)
_CLASSES = [
    (0, 0, 1, 2),
    (1, 0, 1, 2),
    (2, 0, 1, 2),
    (0, 1, -2, 5),
    (1, 1, -2, 5),
    (0, 2, -2, 5),
]


def _role_terms():
    """For each window role (nr, nc) list the 8 cross terms as
    (s, a, plane_k, col_off): value = M[(s,a)][:, k, j + col_off]."""
    out = {}
    for nr in range(R):
        for ncol in range(R):
            tl = []
            for mc in range(R):  # same row, other columns
                if mc != ncol:
                    b = abs(mc - ncol)
                    tl.append((nr, 0, b - 1, min(ncol, mc)))
            for mr in range(R):  # other rows
                if mr == nr:
                    continue
                if mr > nr:
                    for mc in range(R):
                        tl.append((nr, mr - nr, mc - ncol + 2, ncol))
                else:
                    for mc in range(R):
                        tl.append((mr, nr - mr, ncol - mc + 2, mc))
            assert len(tl) == 8
            out[(nr, ncol)] = tl
    return out


def _wide_pair(xs_tile, xa_tile, bmin, nb):
    """APs for one wide sub: anchor broadcast over nb planes, other operand
    shifted by b = bmin..bmin+nb-1 columns per plane."""
    anchor = xs_tile[:, PAD : PAD + W].unsqueeze(1).to_broadcast([ROWS_OUT, nb, W])
    base = xa_tile[:, PAD + bmin : PAD + bmin + W].unsqueeze(1)
    shifted = dataclasses.replace(
        base, ap=[list(base.ap[0]), [1, nb], list(base.ap[2])]
    )
    return anchor, shifted


def _build_program():
    nc = bacc.Bacc("TRN2")
    xin = nc.dram_tensor("xin", (C, ROWS_IN, WT), F32, kind="ExternalInput")
    yout = nc.dram_tensor("yout", (ROWS_OUT, WOUT), F32, kind="ExternalOutput")

    with tile.TileContext(nc) as tc:
        with (
            tc.tile_pool(name="xp", bufs=1) as xp,
            tc.tile_pool(name="mp", bufs=1) as mp,
            tc.tile_pool(name="tp", bufs=2) as tp,
            tc.tile_pool(name="sp", bufs=1) as sp,
        ):
            # ---- stage A: row-shifted input copies (single DMA each) ------
            X = {}
            for c in range(C):
                for s in range(R):
                    t = xp.tile([ROWS_OUT, WT], F32, tag=f"x_{c}_{s}")
                    nc.gpsimd.dma_start(out=t, in_=xin[c, s : s + ROWS_OUT, :])
                    X[(c, s)] = t

            # ---- stage B: E maps, wide over column-gap planes -------------
            M = {}
            for s, a, bmin, nb in _CLASSES:
                mt = mp.tile([ROWS_OUT, nb, W], BF16, tag=f"m_{s}_{a}")
                sq = []
                for c in range(C):
                    d = tp.tile([ROWS_OUT, nb, W], F32, tag=f"d{c}")
                    a0, a1 = _wide_pair(X[(c, s)], X[(c, s + a)], bmin, nb)
                    nc.vector.tensor_sub(d, a0, a1)
                    q = tp.tile([ROWS_OUT, nb, W], F32, tag=f"q{c}")
                    nc.scalar.square(q, d)
                    sq.append(q)
                d2t = tp.tile([ROWS_OUT, nb, W], F32, tag="d2")
                nc.gpsimd.tensor_add(d2t, sq[0], sq[1])
                nc.gpsimd.tensor_add(d2t, d2t, sq[2])
                nc.scalar.activation(mt, d2t, AF.Exp, scale=-2.0)
                M[(s, a)] = mt

            # ---- stage C: per-role S sums ---------------------------------
            def term_ap(t4):
                s, a, k, c0 = t4
                return M[(s, a)][:, k, c0 : c0 + WOUT]

            S = []
            for role, tl in _role_terms().items():
                st = sp.tile([ROWS_OUT, WOUT], BF16, tag=f"s_{role[0]}_{role[1]}")
                nc.vector.scalar_tensor_tensor(
                    out=st,
                    in0=term_ap(tl[0]),
                    scalar=1.0,
                    in1=term_ap(tl[1]),
                    op0=AOP.add,
                    op1=AOP.add,
                )
                for t4 in tl[2:]:
                    nc.vector.tensor_add(st, st, term_ap(t4))
                S.append(st)

            # ---- stage D: product of 9 S maps, log, affine ----------------
            def mul(x, y, tag):
                o = sp.tile([ROWS_OUT, WOUT], BF16, tag=tag)
                nc.vector.tensor_mul(o, x, y)
                return o

            p01 = mul(S[0], S[1], "p01")
            p23 = mul(S[2], S[3], "p23")
            p45 = mul(S[4], S[5], "p45")
            p67 = mul(S[6], S[7], "p67")
            q0 = mul(p01, p23, "q0")
            q1 = mul(p45, p67, "q1")
            q2 = mul(q0, q1, "q2")
            P = mul(q2, S[8], "pp")

            L = sp.tile([ROWS_OUT, WOUT], F32, tag="ln")
            nc.scalar.activation(L, P, AF.Ln)
            OUT = sp.tile([ROWS_OUT, WOUT], F32, tag="out")
            nc.vector.tensor_scalar(
                out=OUT,
                in0=L,
                scalar1=-1.0 / 9.0,
                scalar2=LOG_NORM,
                op0=AOP.mult,
                op1=AOP.add,
            )
            nc.gpsimd.dma_start(out=yout[:, :], in_=OUT)
    if not nc.is_finalized():
        nc.finalize()
    return nc


_PROGRAM = None


def _get_program():
    global _PROGRAM
    if _PROGRAM is None:
        _PROGRAM = _build_program()
    return _PROGRAM


def _shard_inputs(x):
    x = np.asarray(x, dtype=np.float32)
    xp = np.zeros((B, C, 256, WT), dtype=np.float32)
    xp[:, :, :, PAD : PAD + W] = x
    in_maps = []
    for core in range(8):
        b, half = divmod(core, 2)
        r0 = half * 127
        in_maps.append({"xin": np.ascontiguousarray(xp[b, :, r0 : r0 + ROWS_IN, :])})
    return in_maps


def _gather(results):
    out = np.empty((B, 254, 254), dtype=np.float32)
    for core in range(8):
        b, half = divmod(core, 2)
        out[b, half * 127 : half * 127 + 127, :] = results[core]["yout"]
    return out


def kernel(x, **_unused):
    nc = _get_program()
    res = run_bass_kernel_spmd(nc, _shard_inputs(x), core_ids=list(range(8)))
    return _gather(res.results)


def kernel_traced(x):
    """Same as kernel() but returns (output, BassKernelResults) with trace."""
    nc = _get_program()
    res = run_bass_kernel_spmd(
        nc, _shard_inputs(x), core_ids=list(range(8)), trace=True
    )
    return _gather(res.results), res



# revision 7
# speedup vs baseline: 1.7439x; 1.7439x over previous
"""Joint-entropy (KDE logsumexp over 3x3 windows) Trainium2 kernel, v2.

Math: for each 3x3 window of pixel vectors v_n (C=3 channels),
  out[i,j] = log_norm - (1/9) * sum_n log(S_n),  S_n = sum_m exp(-2*||v_n-v_m||^2)
with log_norm = log(9) + 3*log(sqrt(2*pi)*0.5)  (h = 0.5, logits = -2*d2).

Sharding: 8 cores = 4 batches x 2 row-halves. Each core gets a host-padded
bf16 x[b,:,r0:r0+129,:] slice (and a column-shifted copy) and produces a
[127, 254] fp32 output slab. All window math is local; no collectives.

Design (vs v1):
- E-planes are indexed by ABSOLUTE row (partition = input row), deduping the
  per-anchor recomputation: 14 plane-slots instead of 21.
    E0A[p,t,u] = E((p,u),(p,u+t+1))      t in {0,1}   rows 0..127
    E0B[p,t,u] = E((p+1,u),(p+1,u+t+1))  t in {0,1}   rows 1..128
    E1 [p,t,u] = E((p,u),(p+1,u+t-2))    t in 0..4    rows 0..127
    E2 [p,t,u] = E((p,u),(p+2,u+t-2))    t in 0..4    rows 0..126
- Per-role window sums run on the (otherwise idle) TensorEngine: 72
  accumulating matmuls whose stationary is a 0/1 shift-band matrix, so the
  partition-shifted reads are free. Output: 9 role maps in PSUM fp32.
- ScalarE does Ln per role straight out of PSUM; a bf16 add tree and one
  tensor_scalar finish the output. Square/Exp/Ln all live in one ACT table
  set, so only one table load.
- All DMAs go through HWDGE (nc.sync) - GpSimd does no descriptor work.
- bf16 everywhere on-chip except PSUM accumulation; column-shifted input
  copies keep every wide DVE operand 4B-aligned (2x mode).
"""

import dataclasses

import ml_dtypes
import numpy as np

import concourse.bacc as bacc
import concourse.tile as tile
from concourse import mybir
from concourse.bass_utils import run_bass_kernel_spmd

F32 = mybir.dt.float32
BF16 = mybir.dt.bfloat16
AOP = mybir.AluOpType
AF = mybir.ActivationFunctionType

B = 4
C = 3
W = 256
PAD = 2
WT = W + 2 * PAD
ROWS_IN = 129
ROWS_OUT = 127
WOUT = 254
LOG_NORM = float(np.log(9.0) + 3.0 * np.log(np.sqrt(2.0 * np.pi) * 0.5))

# groups: name -> (n_partitions, nb, anchor_tile, partner_tile, partner_shift_tile)
# anchor pixel (p, u); partner (p + da, u + dc).


def _role_terms():
    """Per role (nr, ncol): list of 8 terms (tile_name, s, t, c0).

    Term value for window (i, j) = E<tile>[i + s, t, j + c0]."""
    out = {}
    for nr in range(3):
        for ncol in range(3):
            tl = []
            for mr in range(3):
                for mc in range(3):
                    if (mr, mc) == (nr, ncol):
                        continue
                    if mr == nr:
                        dc = abs(mc - ncol)
                        if nr <= 1:
                            tl.append(("E0A", nr, dc - 1, min(ncol, mc)))
                        else:
                            tl.append(("E0B", 1, dc - 1, min(ncol, mc)))
                    elif mr > nr:
                        a = mr - nr
                        dc = mc - ncol
                        tl.append((f"E{a}", nr if a == 1 else 0, dc + 2, ncol))
                    else:
                        a = nr - mr
                        dc = ncol - mc
                        tl.append((f"E{a}", mr if a == 1 else 0, dc + 2, mc))
            assert len(tl) == 8
            out[(nr, ncol)] = tl
    return out


def _wide(ap3, step, count):
    """Turn a [P, 1, W] AP into [P, count, W] with the middle dim striding
    `step` elements."""
    return dataclasses.replace(
        ap3, ap=[list(ap3.ap[0]), [step, count], list(ap3.ap[2])]
    )


def _build_program():
    nc = bacc.Bacc("TRN2")
    xin = nc.dram_tensor("xin", (C, ROWS_IN, WT), BF16, kind="ExternalInput")
    xin_s = nc.dram_tensor("xin_s", (C, ROWS_IN, WT), BF16, kind="ExternalInput")
    wsh = nc.dram_tensor("wsh", (128, 2, 128), BF16, kind="ExternalInput")
    yout = nc.dram_tensor("yout", (ROWS_OUT, WOUT), F32, kind="ExternalOutput")

    terms = _role_terms()

    with tile.TileContext(nc) as tc:
        with (
            tc.tile_pool(name="xp", bufs=1) as xp,
            tc.tile_pool(name="dp", bufs=1) as dp,
            tc.tile_pool(name="ep", bufs=1) as ep,
            tc.tile_pool(name="pp", bufs=1, space="PSUM") as pp,
            tc.tile_pool(name="sp", bufs=1) as sp,
        ):
            # ---- weights + row-shifted inputs (HWDGE) ---------------------
            WS = xp.tile([128, 2, 128], BF16, tag="wsh")
            nc.sync.dma_start(out=WS, in_=wsh[:, :, :])
            X = {}
            Xs = {}
            for s, p in ((0, 128), (1, 128), (2, 127)):
                X[s] = xp.tile([p, C, WT], BF16, tag=f"x{s}", name=f"x{s}")
                nc.sync.dma_start(out=X[s], in_=xin[:, s : s + p, :].transpose([1, 0, 2]))
                Xs[s] = xp.tile([p, C, WT], BF16, tag=f"xs{s}", name=f"xs{s}")
                nc.sync.dma_start(out=Xs[s], in_=xin_s[:, s : s + p, :].transpose([1, 0, 2]))

            # ---- stage B: E plane groups ---------------------------------
            # (name, P, nb, anchor_s, partner_s)
            groups = [
                ("E0A", 128, 2, 0, 0),
                ("E0B", 128, 2, 1, 1),
                ("E1", 128, 5, 0, 1),
                ("E2", 127, 5, 0, 2),
            ]
            E = {}

            def build_group(name, P, nb, s_a, s_p):
                xa, xb, xbs = X[s_a], X[s_p], Xs[s_p]
                D = [dp.tile([P, nb, W], BF16, tag=f"d_{name}_{c}", name=f"d_{name}_{c}") for c in range(C)]
                for c in range(C):
                    anchor1 = xa[0:P, c, PAD : PAD + W].unsqueeze(1)
                    if nb == 2:
                        # planes: t=0 (dc=+1, from shifted copy), t=1 (dc=+2)
                        nc.vector.tensor_sub(
                            D[c][:, 0, :],
                            xa[0:P, c, PAD : PAD + W],
                            xbs[0:P, c, PAD : PAD + W],
                        )
                        nc.vector.tensor_sub(
                            D[c][:, 1, :],
                            xa[0:P, c, PAD : PAD + W],
                            xb[0:P, c, PAD + 2 : PAD + 2 + W],
                        )
                    else:
                        # even planes t=0,2,4 (dc=-2,0,+2) from xb
                        nc.vector.tensor_sub(
                            _wide(D[c][:, 0:1, :], 2 * W, 3),
                            anchor1.to_broadcast([P, 3, W]),
                            _wide(xb[0:P, c, PAD - 2 : PAD - 2 + W].unsqueeze(1), 2, 3),
                        )
                        # odd planes t=1,3 (dc=-1,+1) from shifted copy
                        nc.vector.tensor_sub(
                            _wide(D[c][:, 1:2, :], 2 * W, 2),
                            anchor1.to_broadcast([P, 2, W]),
                            _wide(
                                xbs[0:P, c, PAD - 2 : PAD - 2 + W].unsqueeze(1), 2, 2
                            ),
                        )
                # d2 = D0^2 + D1^2 + D2^2  (ch0 squared on vector, ch1/2 on scalar)
                q0 = dp.tile([P, nb, W], BF16, tag=f"q0_{name}")
                nc.vector.tensor_mul(q0, D[0], D[0])
                q1 = dp.tile([P, nb, W], BF16, tag=f"q1_{name}")
                nc.scalar.square(q1, D[1])
                q2 = dp.tile([P, nb, W], BF16, tag=f"q2_{name}")
                nc.scalar.square(q2, D[2])
                d2a = dp.tile([P, nb, W], BF16, tag=f"d2a_{name}")
                nc.vector.tensor_add(d2a, q0, q1)
                d2 = dp.tile([P, nb, W], BF16, tag=f"d2_{name}")
                nc.vector.tensor_add(d2, d2a, q2)
                Eg = ep.tile([P, nb, W], BF16, tag=f"e_{name}", name=f"e_{name}")
                nc.scalar.activation(Eg, d2, AF.Exp, scale=-2.0)
                E[name] = Eg

            for g in groups:
                build_group(*g)

            # ---- stage C: role sums on the TensorEngine ------------------
            # 9 roles packed 2-per-PSUM-bank: tile k holds roles 2k, 2k+1.
            S = [
                pp.tile([128, 2, WOUT], F32, tag=f"s{k}", name=f"s{k}")
                for k in range(5)
            ]
            # PSUM accumulation groups are bank-granular: start=True zeroes
            # the whole 2KB zero-region. One group per bank => start on the
            # first matmul touching the bank, stop on the last.
            started = set()
            order = []
            for tname, _, _, _, _ in groups:
                for role, tl in terms.items():
                    for term in tl:
                        if term[0] == tname:
                            order.append((role, term))
            last_idx = {}
            for idx, (role, _) in enumerate(order):
                last_idx[role[0] * 3 + role[1] >> 1] = idx
            group_p = {g[0]: g[1] for g in groups}
            for idx, (role, (tname, s, t, c0)) in enumerate(order):
                r = role[0] * 3 + role[1]
                Eg = E[tname]
                k = group_p[tname]
                lhsT = WS[0:k, s, :]
                rhs = Eg[0:k, t, c0 : c0 + WOUT]
                nc.tensor.matmul(
                    S[r // 2][:, r % 2, :],
                    lhsT,
                    rhs,
                    start=(r // 2 not in started),
                    stop=(idx == last_idx[r // 2]),
                    skip_group_check=True,
                )
                started.add(r // 2)

            # ---- stage D: ln, sum, affine --------------------------------
            LT = sp.tile([ROWS_OUT, 9, WOUT], BF16, tag="lt")
            for r in range(9):
                nc.scalar.activation(
                    LT[:, r, :], S[r // 2][0:ROWS_OUT, r % 2, :], AF.Ln, bias=1.0
                )
            # pairwise tree: 0+=1, 2+=3, 4+=5, 6+=7; 0+=2, 4+=6; 0+=4; 0+=8
            for a, b_ in ((0, 1), (2, 3), (4, 5), (6, 7), (0, 2), (4, 6), (0, 4), (0, 8)):
                nc.vector.tensor_add(LT[:, a, :], LT[:, a, :], LT[:, b_, :])
            OUT = sp.tile([ROWS_OUT, WOUT], F32, tag="out")
            nc.vector.tensor_scalar(
                out=OUT,
                in0=LT[:, 0, :],
                scalar1=-1.0 / 9.0,
                scalar2=LOG_NORM,
                op0=AOP.mult,
                op1=AOP.add,
            )
            nc.sync.dma_start(out=yout[:, :], in_=OUT)
    if not nc.is_finalized():
        nc.finalize()
    return nc


_PROGRAM = None


def _get_program():
    global _PROGRAM
    if _PROGRAM is None:
        _PROGRAM = _build_program()
    return _PROGRAM


def _make_shift_weights():
    w = np.zeros((128, 2, 128), dtype=ml_dtypes.bfloat16)
    for s in range(2):
        for m in range(128):
            if m + s < 128:
                w[m + s, s, m] = 1.0
    return w


def _shard_inputs(x):
    x = np.asarray(x, dtype=np.float32)
    xp = np.zeros((B, C, 256, WT), dtype=np.float32)
    xp[:, :, :, PAD : PAD + W] = x
    xs = np.zeros_like(xp)
    xs[:, :, :, : WT - 1] = xp[:, :, :, 1:]
    xp16 = xp.astype(ml_dtypes.bfloat16)
    xs16 = xs.astype(ml_dtypes.bfloat16)
    wsh = _make_shift_weights()
    in_maps = []
    for core in range(8):
        b, half = divmod(core, 2)
        r0 = half * 127
        in_maps.append(
            {
                "xin": np.ascontiguousarray(xp16[b, :, r0 : r0 + ROWS_IN, :]),
                "xin_s": np.ascontiguousarray(xs16[b, :, r0 : r0 + ROWS_IN, :]),
                "wsh": wsh,
            }
        )
    return in_maps


def _gather(results):
    out = np.empty((B, 254, 254), dtype=np.float32)
    for core in range(8):
        b, half = divmod(core, 2)
        out[b, half * 127 : half * 127 + 127, :] = results[core]["yout"]
    return out


def kernel(x, **_unused):
    nc = _get_program()
    res = run_bass_kernel_spmd(nc, _shard_inputs(x), core_ids=list(range(8)))
    return _gather(res.results)


def kernel_traced(x):
    """Same as kernel() but returns (output, BassKernelResults) with trace."""
    nc = _get_program()
    res = run_bass_kernel_spmd(
        nc, _shard_inputs(x), core_ids=list(range(8)), trace=True
    )
    return _gather(res.results), res


# revision 8
# speedup vs baseline: 2.6635x; 1.5274x over previous
"""Joint-entropy (KDE logsumexp over 3x3 windows) Trainium2 kernel, v3.

Math: for each 3x3 window of pixel vectors v_n (C=3 channels),
  out[i,j] = log_norm - (1/9) * sum_n log(S_n),  S_n = sum_m exp(-2*||v_n-v_m||^2)
with log_norm = log(9) + 3*log(sqrt(2*pi)*0.5)  (h = 0.5, logits = -2*d2).

Sharding: 8 cores = 4 batches x 2 row-halves. Each core gets a host-padded
bf16 x[b,:,r0:r0+130,:] slice (row-major [rows, C, W], and a column-shifted
copy) and produces a [128, 254] fp32 output slab (row 127 garbage, host
drops it). All window math is local; no collectives.

Design:
- E-planes indexed by ABSOLUTE row (partition = input row): 14 plane-slots.
    E0A[p,t,u] = E((p,u),(p,u+t+1))      t in {0,1}   rows 0..127
    E0B[p,t,u] = E((p+1,u),(p+1,u+t+1))  t in {0,1}   rows 1..128
    E1 [p,t,u] = E((p,u),(p+1,u+t-2))    t in 0..4    rows 0..127
    E2 [p,t,u] = E((p,u),(p+2,u+t-2))    t in 0..4    rows 0..126
- Per-role window sums run on the TensorEngine: 72 accumulating matmuls
  whose stationary is a 0/1 shift-band matrix (partition-shifted reads are
  free). 9 role maps in PSUM fp32, 2 roles per PSUM bank (accumulation
  groups are bank-granular: one start/stop per bank).
- ScalarE runs Ln(1 + S) per role straight out of PSUM (the +1 self term
  rides the ACT affine); bf16 add tree + one tensor_scalar finish.
- Square/Exp/Ln forced into ONE ACT table set (natural_log_exp_and_others)
  so there is a single table load.
- Every DMA moves a 128-partition pattern (HWDGE only fans out across the
  16 DMA engines for 128-partition patterns) with 1560B-contiguous rows,
  split across the SP and ACT HWDGE queues.
"""

import dataclasses

import ml_dtypes
import numpy as np

import concourse.bacc as bacc
import concourse.tile as tile
from concourse import mybir
from concourse.bass_utils import run_bass_kernel_spmd

F32 = mybir.dt.float32
BF16 = mybir.dt.bfloat16
AOP = mybir.AluOpType
AF = mybir.ActivationFunctionType

B = 4
C = 3
W = 256
PAD = 2
WT = W + 2 * PAD
ROWS_IN = 130  # 129 real rows + 1 zero pad row so X2 can be 128 partitions
ROWS_OUT = 127
WOUT = 254
LOG_NORM = float(np.log(9.0) + 3.0 * np.log(np.sqrt(2.0 * np.pi) * 0.5))

# role r = nr*3 + ncol -> (psum bank, slot). Banks are paired so that the
# (nr=1) roles 3,4 share a bank whose accumulation finishes with the E1
# matmul block (their Lns overlap E2 compute); all other banks finish in
# the E2 block.
ROLE_SLOT = {
    3: (0, 0), 4: (0, 1),
    5: (1, 0), 0: (1, 1),
    1: (2, 0), 2: (2, 1),
    6: (3, 0), 7: (3, 1),
    8: (4, 0),
}


def _role_terms():
    """Per role (nr, ncol): list of 8 terms (tile_name, s, t, c0).

    Term value for window (i, j) = E<tile>[i + s, t, j + c0]."""
    out = {}
    for nr in range(3):
        for ncol in range(3):
            tl = []
            for mr in range(3):
                for mc in range(3):
                    if (mr, mc) == (nr, ncol):
                        continue
                    if mr == nr:
                        dc = abs(mc - ncol)
                        if nr <= 1:
                            tl.append(("E0A", nr, dc - 1, min(ncol, mc)))
                        else:
                            tl.append(("E0B", 1, dc - 1, min(ncol, mc)))
                    elif mr > nr:
                        a = mr - nr
                        dc = mc - ncol
                        tl.append((f"E{a}", nr if a == 1 else 0, dc + 2, ncol))
                    else:
                        a = nr - mr
                        dc = ncol - mc
                        tl.append((f"E{a}", mr if a == 1 else 0, dc + 2, mc))
            assert len(tl) == 8
            out[(nr, ncol)] = tl
    return out


def _wide(ap3, step, count):
    """Turn a [P, 1, W] AP into [P, count, W] with the middle dim striding
    `step` elements."""
    return dataclasses.replace(
        ap3, ap=[list(ap3.ap[0]), [step, count], list(ap3.ap[2])]
    )


class _one_act_table:
    """Force Square/Exp/Ln to resolve to natural_log_exp_and_others so the
    kernel needs a single ACT table load. Set membership is edited in-place
    on a copy; set order/ids are preserved."""

    WANT = "natural_log_exp_and_others"
    FNS = frozenset({AF.Exp, AF.Ln, AF.Square})

    def __enter__(self):
        self._orig = bacc.get_activation_tables

        def patched(arch, _orig=self._orig):
            tabs = dict(_orig(arch))
            if self.WANT in tabs and self.FNS <= tabs[self.WANT]:
                tabs = {
                    k: (v if k == self.WANT else set(v) - self.FNS)
                    for k, v in tabs.items()
                }
            return tabs

        bacc.get_activation_tables = patched
        return self

    def __exit__(self, *exc):
        bacc.get_activation_tables = self._orig
        return False


def _build_program():
    nc = bacc.Bacc("TRN2")
    xin = nc.dram_tensor("xin", (ROWS_IN, C, WT), BF16, kind="ExternalInput")
    xin_s = nc.dram_tensor("xin_s", (ROWS_IN, C, WT), BF16, kind="ExternalInput")
    wsh = nc.dram_tensor("wsh", (128, 2, 128), BF16, kind="ExternalInput")
    yout = nc.dram_tensor("yout", (128, WOUT), F32, kind="ExternalOutput")

    terms = _role_terms()

    with tile.TileContext(nc) as tc:
        with (
            tc.tile_pool(name="xp", bufs=1) as xp,
            tc.tile_pool(name="dp", bufs=1) as dp,
            tc.tile_pool(name="ep", bufs=1) as ep,
            tc.tile_pool(name="pp", bufs=1, space="PSUM") as pp,
            tc.tile_pool(name="sp", bufs=1) as sp,
        ):
            # ---- weights + row-shifted inputs (HWDGE, 2 queues) ----------
            WS = xp.tile([128, 2, 128], BF16, tag="wsh")
            nc.scalar.dma_start(out=WS, in_=wsh[:, :, :])
            X = {}
            Xs = {}
            for s in (0, 1, 2):
                X[s] = xp.tile([128, C, WT], BF16, tag=f"x{s}", name=f"x{s}")
                nc.sync.dma_start(out=X[s], in_=xin[s : s + 128, :, :])
                Xs[s] = xp.tile([128, C, WT], BF16, tag=f"xs{s}", name=f"xs{s}")
                nc.scalar.dma_start(out=Xs[s], in_=xin_s[s : s + 128, :, :])

            # ---- stage B: E plane groups ---------------------------------
            # (name, P, nb, anchor_s, partner_s)
            groups = [
                ("E0A", 128, 2, 0, 0),
                ("E0B", 128, 2, 1, 1),
                ("E1", 128, 5, 0, 1),
                ("E2", 127, 5, 0, 2),
            ]
            E = {}

            def build_group(name, P, nb, s_a, s_p):
                xa, xb, xbs = X[s_a], X[s_p], Xs[s_p]
                D = [
                    dp.tile([P, nb, W], BF16, tag=f"d_{name}_{c}",
                            name=f"d_{name}_{c}")
                    for c in range(C)
                ]
                for c in range(C):
                    anchor1 = xa[0:P, c, PAD : PAD + W].unsqueeze(1)
                    if nb == 2:
                        nc.vector.tensor_sub(
                            D[c][:, 0, :],
                            xa[0:P, c, PAD : PAD + W],
                            xbs[0:P, c, PAD : PAD + W],
                        )
                        nc.vector.tensor_sub(
                            D[c][:, 1, :],
                            xa[0:P, c, PAD : PAD + W],
                            xb[0:P, c, PAD + 2 : PAD + 2 + W],
                        )
                    else:
                        # even planes t=0,2,4 (dc=-2,0,+2) from xb
                        nc.vector.tensor_sub(
                            _wide(D[c][:, 0:1, :], 2 * W, 3),
                            anchor1.to_broadcast([P, 3, W]),
                            _wide(xb[0:P, c, PAD - 2 : PAD - 2 + W].unsqueeze(1), 2, 3),
                        )
                        # odd planes t=1,3 (dc=-1,+1) from shifted copy
                        nc.vector.tensor_sub(
                            _wide(D[c][:, 1:2, :], 2 * W, 2),
                            anchor1.to_broadcast([P, 2, W]),
                            _wide(
                                xbs[0:P, c, PAD - 2 : PAD - 2 + W].unsqueeze(1), 2, 2
                            ),
                        )
                # d2 = D0^2 + D1^2 + D2^2 (ch0 on vector, ch1/2 on scalar)
                q0 = dp.tile([P, nb, W], BF16, tag=f"q0_{name}")
                nc.vector.tensor_mul(q0, D[0], D[0])
                q1 = dp.tile([P, nb, W], BF16, tag=f"q1_{name}")
                nc.scalar.square(q1, D[1])
                q2 = dp.tile([P, nb, W], BF16, tag=f"q2_{name}")
                nc.scalar.square(q2, D[2])
                d2a = dp.tile([P, nb, W], BF16, tag=f"d2a_{name}")
                nc.vector.tensor_add(d2a, q0, q1)
                d2 = dp.tile([P, nb, W], BF16, tag=f"d2_{name}")
                nc.vector.tensor_add(d2, d2a, q2)
                Eg = ep.tile([P, nb, W], BF16, tag=f"e_{name}", name=f"e_{name}")
                nc.scalar.activation(Eg, d2, AF.Exp, scale=-2.0)
                E[name] = Eg

            for g in groups:
                build_group(*g)

            # ---- stage C: role sums on the TensorEngine ------------------
            S = [
                pp.tile([128, 2, WOUT], F32, tag=f"s{k}", name=f"s{k}")
                for k in range(5)
            ]
            started = set()
            order = []
            for tname, _, _, _, _ in groups:
                for role, tl in terms.items():
                    for term in tl:
                        if term[0] == tname:
                            order.append((role, term))
            last_idx = {}
            for idx, (role, _) in enumerate(order):
                last_idx[ROLE_SLOT[role[0] * 3 + role[1]][0]] = idx
            group_p = {g[0]: g[1] for g in groups}
            for idx, (role, (tname, s, t, c0)) in enumerate(order):
                bank, slot = ROLE_SLOT[role[0] * 3 + role[1]]
                Eg = E[tname]
                k = group_p[tname]
                nc.tensor.matmul(
                    S[bank][:, slot, :],
                    WS[0:k, s, :],
                    Eg[0:k, t, c0 : c0 + WOUT],
                    start=(bank not in started),
                    stop=(idx == last_idx[bank]),
                    skip_group_check=True,
                )
                started.add(bank)

            # ---- stage D: ln, sum, affine (full 128 partitions) ----------
            LT = sp.tile([128, 9, WOUT], BF16, tag="lt")
            for r in range(9):
                bank, slot = ROLE_SLOT[r]
                nc.scalar.activation(LT[:, r, :], S[bank][:, slot, :], AF.Ln, bias=1.0)
            for a, b_ in ((0, 1), (2, 3), (4, 5), (6, 7), (0, 2), (4, 6), (0, 4), (0, 8)):
                nc.vector.tensor_add(LT[:, a, :], LT[:, a, :], LT[:, b_, :])
            OUT = sp.tile([128, WOUT], F32, tag="out")
            nc.vector.tensor_scalar(
                out=OUT,
                in0=LT[:, 0, :],
                scalar1=-1.0 / 9.0,
                scalar2=LOG_NORM,
                op0=AOP.mult,
                op1=AOP.add,
            )
            nc.sync.dma_start(out=yout[:, :], in_=OUT)
    if not nc.is_finalized():
        with _one_act_table():
            nc.finalize()
    return nc


_PROGRAM = None


def _get_program():
    global _PROGRAM
    if _PROGRAM is None:
        _PROGRAM = _build_program()
    return _PROGRAM


def _make_shift_weights():
    w = np.zeros((128, 2, 128), dtype=ml_dtypes.bfloat16)
    for s in range(2):
        for m in range(128):
            if m + s < 128:
                w[m + s, s, m] = 1.0
    return w


def _shard_inputs(x):
    x = np.asarray(x, dtype=np.float32)
    # padded, row-major [B, rows, C, WT] with one zero pad row at the end
    xp = np.zeros((B, 257, C, WT), dtype=np.float32)
    xp[:, :256, :, PAD : PAD + W] = x.transpose(0, 2, 1, 3)
    xs = np.zeros_like(xp)
    xs[:, :, :, : WT - 1] = xp[:, :, :, 1:]
    xp16 = xp.astype(ml_dtypes.bfloat16)
    xs16 = xs.astype(ml_dtypes.bfloat16)
    wsh = _make_shift_weights()
    in_maps = []
    for core in range(8):
        b, half = divmod(core, 2)
        r0 = half * 127
        in_maps.append(
            {
                "xin": np.ascontiguousarray(xp16[b, r0 : r0 + ROWS_IN]),
                "xin_s": np.ascontiguousarray(xs16[b, r0 : r0 + ROWS_IN]),
                "wsh": wsh,
            }
        )
    return in_maps


def _gather(results):
    out = np.empty((B, 254, 254), dtype=np.float32)
    for core in range(8):
        b, half = divmod(core, 2)
        out[b, half * 127 : half * 127 + 127, :] = results[core]["yout"][:127]
    return out


def kernel(x, **_unused):
    nc = _get_program()
    res = run_bass_kernel_spmd(nc, _shard_inputs(x), core_ids=list(range(8)))
    return _gather(res.results)


def kernel_traced(x):
    """Same as kernel() but returns (output, BassKernelResults) with trace."""
    nc = _get_program()
    res = run_bass_kernel_spmd(
        nc, _shard_inputs(x), core_ids=list(range(8)), trace=True
    )
    return _gather(res.results), res
